# revision 29
# baseline (speedup 1.0000x reference)
"""Multi-head attention (B=2, S=2048, H=16, D=64) with QKV projection,
softmax and deterministic dropout, on 8 Trainium2 NeuronCores.

Sharding: batch*heads (32 pairs) -> 4 pairs per core (data+head parallel).

Per-core algorithm (per (b,h) pair), using the softmax shift-invariance to
fold the projections:
    scores = (q_in Wq + bq)(k_in Wk + bk)^T / 8
           = q_in A8 k_in^T + (row-const terms, dropped) + v8[j]
    A8 = Wq Wk^T / 8,  v8 = k_in (Wk bq) / 8
Everything is computed in the transposed layout scoresT[j, i] so that the
attention @ V matmul needs no transposes:
    - kinT/qinT via PE transposes; qaT = A8^T-proj of qinT (fp32r matmuls)
    - scoresT tiles = kinT-chunk.T @ qaT (fp32r, PSUM)
    - eT = exp(scoresT + v8[j]) on ScalarE (fp16, bias = per-partition AP)
    - uT = eT * dropout_keep (DVE, mask streamed as fp8 -> fp16 cast DMA)
    - row sums S_i: matmul(lhsT=eT-tile, rhs=ones[128,1]) accumulated in PSUM
    - out tiles:    matmul(lhsT=uT-tile, rhs=v-tile[128,64]) accumulated
    - final: out * (1 / (0.9 * S_i)) per-partition scalar, fp16, DMA out
"""

import sys

sys.path.insert(0, "/opt/trn_rl_repo")

import functools
import math

import numpy as np
import ml_dtypes

B, S, H, D = 2, 2048, 16, 64
N_CORES = 8
HPC = (B * H) // N_CORES  # 4 (b,h) pairs per core
NT = S // 128  # 16 seq tiles
DROP_P = 0.1


@functools.lru_cache(maxsize=1)
def _keep_mask():
    """The dropout keep-mask, bit-exact with the reference (jax threefry)."""
    import jax

    with jax.default_device(jax.devices("cpu")[0]):
        keep = jax.random.bernoulli(
            jax.random.key(12345), 1.0 - DROP_P, (B, H, S, S)
        )
        return np.asarray(keep)


@functools.lru_cache(maxsize=1)
def _build():
    from contextlib import ExitStack

    import concourse.bass as bass
    import concourse.mybir as mybir
    import concourse.tile as tile
    from concourse import bacc

    F32 = mybir.dt.float32
    F32R = mybir.dt.float32r
    F16 = mybir.dt.float16
    F8 = mybir.dt.float8e4
    EXP = mybir.ActivationFunctionType.Exp

    nc = bacc.Bacc("TRN2", target_bir_lowering=False, debug=False,
                   num_devices=N_CORES)

    qin_d = nc.declare_dram_parameter("qin", [S, HPC, D], F32, isOutput=False)
    kin_d = nc.declare_dram_parameter("kin", [S, HPC, D], F32, isOutput=False)
    vin_d = nc.declare_dram_parameter("vin", [S, HPC, D], F32, isOutput=False)
    a8_d = nc.declare_dram_parameter("a8", [D, D], F32, isOutput=False)
    w8_d = nc.declare_dram_parameter("w8", [D, 2], F32, isOutput=False)
    wv_d = nc.declare_dram_parameter("wv", [D + 1, D], F32, isOutput=False)
    ident_d = nc.declare_dram_parameter("ident", [128, 128], F32, isOutput=False)
    ones_d = nc.declare_dram_parameter("ones", [128, 1], F16, isOutput=False)
    mask_d = nc.declare_dram_parameter("maskt", [HPC, NT, 128, S], F8,
                                       isOutput=False)
    out_d = nc.declare_dram_parameter("out", [HPC, S, D], F16, isOutput=True)

    with tile.TileContext(nc) as tc, ExitStack() as ctx:
        cst = ctx.enter_context(tc.tile_pool(name="cst", bufs=1))
        inp = ctx.enter_context(tc.tile_pool(name="inp", bufs=1))
        prp = ctx.enter_context(tc.tile_pool(name="prp", bufs=2))
        stp = ctx.enter_context(tc.tile_pool(name="stp", bufs=8))
        msp = ctx.enter_context(tc.tile_pool(name="msp", bufs=6))
        pst = ctx.enter_context(tc.tile_pool(name="pst", bufs=1, space="PSUM"))
        psc = ctx.enter_context(tc.tile_pool(name="psc", bufs=2, space="PSUM"))
        pso = ctx.enter_context(tc.tile_pool(name="pso", bufs=1, space="PSUM"))

        # constants
        ident = cst.tile([128, 128], F32, tag="ident")
        nc.sync.dma_start(ident[:], ident_d[:])
        ones = cst.tile([128, 1], F16, tag="ones")
        nc.sync.dma_start(ones[:], ones_d[:])
        a8f = cst.tile([D, D], F32, tag="a8f")
        nc.sync.dma_start(a8f[:], a8_d[:])
        a8r = cst.tile([D, D], F32R, tag="a8r")
        nc.vector.tensor_copy(a8r[:], a8f[:])
        w8f = cst.tile([D, 2], F32, tag="w8f")
        nc.sync.dma_start(w8f[:], w8_d[:])
        w8r = cst.tile([D, 2], F32R, tag="w8r")
        nc.vector.tensor_copy(w8r[:], w8f[:])
        wv = cst.tile([D + 1, D], F32, tag="wv")
        nc.sync.dma_start(wv[:], wv_d[:])
        warm = cst.tile([D, 2], F16, tag="warm")
        nc.scalar.activation(warm[:], a8f[:, 0:2], EXP)  # pull ACT table load early

        # whole-core input loads: [S, 4, 64] -> [128, t(16) h(4) d(64)]
        qin = inp.tile([128, NT * HPC * D], F32, tag="qin")
        kin = inp.tile([128, NT * HPC * D], F32, tag="kin")
        vin = inp.tile([128, NT * HPC * D], F32, tag="vin")
        for sb_t, dr in ((kin, kin_d), (qin, qin_d), (vin, vin_d)):
            nc.sync.dma_start(
                sb_t[:].rearrange("p (t h d) -> p t h d", t=NT, h=HPC),
                dr.ap().rearrange("(t p) h d -> p t h d", p=128),
            )

        def phase1(p):
            pp = psc if p == 0 else pst  # pair 0: borrow idle score-psum slots
            # ---------------- phase 1: transposes + projections -------------
            qinT = prp.tile([64, S], F32R, tag="qinT")
            kinT = prp.tile([64, S], F32R, tag="kinT")
            vinT = prp.tile([65, S], F32, tag="vinT")
            nc.gpsimd.memset(vinT[64:65, :], 1.0)

            for src, dst in ((kin, kinT), (qin, qinT), (vin, vinT)):
                for tg in range(4):  # groups of 4 transposes
                    tp = pp.tile([64, 512], F32, tag="trps" if pp is pst else "sc")
                    for k in range(4):
                        t = tg * 4 + k
                        nc.tensor.transpose(
                            tp[:, k * 128:(k + 1) * 128],
                            src[:, t * (HPC * D) + p * D: t * (HPC * D) + (p + 1) * D],
                            ident[:],
                        )
                    if dst is vinT:
                        nc.vector.tensor_copy(
                            dst[0:64, tg * 512:(tg + 1) * 512], tp[:])
                    else:
                        nc.vector.tensor_copy(
                            dst[:, tg * 512:(tg + 1) * 512], tp[:])

            # v8[j] = kin[j,:] @ w8  -> per-partition bias [128, NT]
            v8ps = pp.tile([128, 2 * NT], F32, tag="trps" if pp is pst else "sc")
            for jt in range(NT):
                nc.tensor.matmul(
                    v8ps[:, 2 * jt:2 * jt + 2],
                    kinT[:, jt * 128:(jt + 1) * 128],
                    w8r[:],
                    start=True, stop=True,
                )
            v8sb = prp.tile([128, 2 * NT], F32, tag="v8sb")
            nc.vector.tensor_copy(v8sb[:], v8ps[:])

            # qaT = A8.T-projection of qinT  (fp32r)
            qaT = prp.tile([64, S], F32R, tag="qaT")
            for c in range(4):
                qp = pp.tile([64, 512], F32, tag="trps" if pp is pst else "sc")
                nc.tensor.matmul(
                    qp[:],
                    a8r[:],
                    qinT[:, c * 512:(c + 1) * 512],
                    start=True, stop=True,
                )
                nc.vector.tensor_copy(qaT[:, c * 512:(c + 1) * 512], qp[:])

            # v projection: v = vin @ Wv + bv  (natural layout, fp16)
            vsb = prp.tile([128, NT * D], F16, tag="vsb")
            for vg in range(2):
                vp = pp.tile([128, 512], F32, tag="trps" if pp is pst else "sc")
                for k in range(8):
                    jt = vg * 8 + k
                    nc.tensor.matmul(
                        vp[:, k * 64:(k + 1) * 64],
                        vinT[:, jt * 128:(jt + 1) * 128],
                        wv[:],
                        start=True, stop=True,
                    )
                nc.vector.tensor_copy(vsb[:, vg * 512:(vg + 1) * 512], vp[:])
            return kinT, qaT, v8sb, vsb

        ph = phase1(0)
        for p in range(HPC):
            kinT, qaT, v8sb, vsb = ph
            if p + 1 < HPC:
                ph = phase1(p + 1)  # emitted early: overlaps this pair's strips
            # ---------------- phase 2: strips over j-tiles -------------------
            acc = pso.tile([128, NT * D + NT], F32, tag="acc")
            nc.vector.memset(acc[:], 0.0)
            ops = acc[:, 0:NT * D]        # out accum
            sps = acc[:, NT * D:NT * D + NT]  # rowsum accum
            for jt in range(NT):
                msk = msp.tile([128, S], F16, tag="msk")
                nc.gpsimd.dma_start(msk[:], mask_d[p, jt])  # fp8 -> fp16 cast
                for hf in range(2):
                    sc = psc.tile([128, 1024], F32, tag="sc")
                    for k in range(2):
                        nc.tensor.matmul(
                            sc[:, k * 512:(k + 1) * 512],
                            kinT[:, jt * 128:(jt + 1) * 128],
                            qaT[:, hf * 1024 + k * 512: hf * 1024 + (k + 1) * 512],
                            start=True, stop=True,
                        )
                    eT = stp.tile([128, 1024], F16, tag="eT")
                    nc.scalar.activation(eT[:], sc[:], EXP,
                                         bias=v8sb[:, 2 * jt:2 * jt + 1], scale=1.0)
                    uT = stp.tile([128, 1024], F16, tag="uT")
                    nc.vector.tensor_tensor(
                        out=uT[:], in0=eT[:],
                        in1=msk[:, hf * 1024:(hf + 1) * 1024],
                        op=mybir.AluOpType.mult,
                    )
                    for t in range(8):
                        it = hf * 8 + t
                        nc.tensor.matmul(
                            sps[:, it:it + 1],
                            eT[:, t * 128:(t + 1) * 128],
                            ones[:],
                            start=False, stop=(jt == NT - 1),
                            skip_group_check=True,
                        )
                        nc.tensor.matmul(
                            ops[:, it * D:(it + 1) * D],
                            uT[:, t * 128:(t + 1) * 128],
                            vsb[:, jt * D:(jt + 1) * D],
                            start=False, stop=(jt == NT - 1),
                            skip_group_check=True,
                        )

            # ---------------- phase 3: normalize + store ---------------------
            asb = prp.tile([128, NT * D + NT], F32, tag="asb")
            nc.vector.tensor_copy(asb[:], acc[:])
            ssb = asb[:, NT * D:NT * D + NT]
            isb = prp.tile([128, NT], F32, tag="isb")
            nc.scalar.mul(ssb[:], ssb[:], 1.0 - DROP_P)   # 0.9 * S
            nc.vector.reciprocal(isb[:], ssb[:])
            osb = prp.tile([128, NT * D], F16, tag="osb")
            for it in range(NT):
                nc.vector.tensor_scalar_mul(
                    osb[:, it * D:(it + 1) * D],
                    asb[:, it * D:(it + 1) * D],
                    isb[:, it:it + 1],
                )
            nc.sync.dma_start(
                out_d[p].rearrange("(t p) d -> p t d", p=128),
                osb[:].rearrange("p (t d) -> p t d", t=NT),
            )

    nc.compile()
    return nc


def kernel(query, key, value, Wq, bq, Wk, bk, Wv, bv):
    from concourse.bass_utils import run_bass_kernel_spmd

    query = np.asarray(query, dtype=np.float32)
    key = np.asarray(key, dtype=np.float32)
    value = np.asarray(value, dtype=np.float32)
    Wq = np.asarray(Wq, dtype=np.float32)
    bq = np.asarray(bq, dtype=np.float32)
    Wk = np.asarray(Wk, dtype=np.float32)
    bk = np.asarray(bk, dtype=np.float32)
    Wv = np.asarray(Wv, dtype=np.float32)
    bv = np.asarray(bv, dtype=np.float32)

    a8 = (Wq @ Wk.T / math.sqrt(D)).astype(np.float32)          # [64, 64]
    w8 = np.zeros((D, 2), dtype=np.float32)
    w8[:, 0] = (Wk @ bq) / math.sqrt(D)
    wv_aug = np.concatenate([Wv, bv[None, :]], 0).astype(np.float32)  # [65, 64]
    ident = np.eye(128, dtype=np.float32)
    ones = np.ones((128, 1), dtype=np.float16)
    keep = _keep_mask()  # [B, H, S, S] bool

    nc = _build()

    in_maps = []
    for c in range(N_CORES):
        b = c // (H // HPC * B // B)  # 4 cores per batch entry
        b = c // 4
        h0 = (c % 4) * HPC
        # maskT[p, jt, j, i] = keep[b, h0+p, i, jt*128+j]
        km = keep[b, h0:h0 + HPC]                    # [4, S_i, S_j]
        km = np.ascontiguousarray(km.transpose(0, 2, 1))  # [4, S_j, S_i]
        km = km.reshape(HPC, NT, 128, S).astype(ml_dtypes.float8_e4m3)
        in_maps.append({
            "qin": np.ascontiguousarray(query[b, :, h0:h0 + HPC, :]),
            "kin": np.ascontiguousarray(key[b, :, h0:h0 + HPC, :]),
            "vin": np.ascontiguousarray(value[b, :, h0:h0 + HPC, :]),
            "a8": a8, "w8": w8, "wv": wv_aug,
            "ident": ident, "ones": ones,
            "maskt": km,
        })

    res = run_bass_kernel_spmd(nc, in_maps, list(range(N_CORES)))

    out = np.empty((B, H, S, D), dtype=np.float16)
    for c in range(N_CORES):
        b = c // 4
        h0 = (c % 4) * HPC
        out[b, h0:h0 + HPC] = res.results[c]["out"]
    return out
